# revision 40
# baseline (speedup 1.0000x reference)
"""Trainium2 Bass kernel for nn_Attention (B=2, T=2048, E=1024, H=16, D=64).

Sharding: 2 heads per core across 8 cores (tensor-parallel over heads).
Each core computes Q/K/V projections for its 2 heads, causal attention,
and a partial out-projection (its 128 feature columns of Wo); the host
sums the 8 partial outputs.

Precision: f32r (TF32-like, 11-bit mantissa) for projections, QK^T and
out-projection; bf16 for P (attention weights) and PV.

Softmax is streamed per 512-column chunk: each chunk's S tile lives in
one PSUM bank, gets its own (negated) rowmax and exp, and is released
immediately. The chunk-vs-row max fixup factors f_c = exp(m_c - m) are
folded into the P-transpose matmuls as diagonal right-operands; the
final 1/l row normalization is applied to the tiny A^T via a
PE-replicated reciprocal.
"""

import sys

sys.path.insert(0, "/opt/trn_rl_repo")

import numpy as np
import concourse.bass as bass
import concourse.mybir as mybir
import concourse.tile as tile
from concourse import bacc
from concourse import bass_utils
from concourse.masks import make_identity

f32 = mybir.dt.float32
f32r = mybir.dt.float32r
fp16 = mybir.dt.float16
bf16 = mybir.dt.bfloat16
AF = mybir.ActivationFunctionType
ALU = mybir.AluOpType
AX = mybir.AxisListType

B, T, E, H, D = 2, 2048, 1024, 16, 64
HL = 2              # heads per core
F = HL * D          # local feature cols (128)
NT = T // 128       # 16 t-tiles per batch
NE = E // 128       # 8 e-tiles
N_CORES = 8
INV_S = 1.0 / float(np.sqrt(T))


def _round_tf32(a):
    a = np.ascontiguousarray(a, dtype=np.float32)
    u = a.view(np.uint32).astype(np.uint64)
    r = ((u + 0x800 + ((u >> 12) & 1)) & 0xFFFFF000).astype(np.uint32)
    return r.view(np.float32)


def build_nc():
    nc = bacc.Bacc("TRN2", target_bir_lowering=False, debug=False,
                   num_devices=N_CORES)
    xt_d = nc.dram_tensor("xt", [B, E, T], fp16, kind="ExternalInput").ap()
    wq_d = nc.dram_tensor("wq", [E, F], fp16, kind="ExternalInput").ap()
    wk_d = nc.dram_tensor("wk", [E, F], fp16, kind="ExternalInput").ap()
    wv_d = nc.dram_tensor("wv", [E, F], bf16, kind="ExternalInput").ap()
    wot_d = nc.dram_tensor("wot", [F, E], fp16, kind="ExternalInput").ap()
    xtb_d = nc.dram_tensor("xtb", [B, E, T], bf16, kind="ExternalInput").ap()
    out_d = nc.dram_tensor("out", [B, T, E], bf16, kind="ExternalOutput").ap()

    with tile.TileContext(nc) as tc:
        with tc.tile_pool(name="const", bufs=1) as cpool, \
             tc.tile_pool(name="xtp", bufs=1) as xtp, \
             tc.tile_pool(name="qkv", bufs=2) as qkvp, \
             tc.tile_pool(name="pp", bufs=4) as ppool, \
             tc.tile_pool(name="pts", bufs=12) as ptsp, \
             tc.tile_pool(name="dgp", bufs=4) as dgp, \
             tc.tile_pool(name="smallp", bufs=6) as smallp, \
             tc.tile_pool(name="outp", bufs=6) as outp, \
             tc.tile_pool(name="ps_s", bufs=4, space="PSUM") as ps_s, \
             tc.tile_pool(name="ps_pt", bufs=2, space="PSUM") as ps_pt, \
             tc.tile_pool(name="ps_a", bufs=1, space="PSUM") as ps_a, \
             tc.tile_pool(name="ps_o", bufs=1, space="PSUM") as ps_o:

            # ---- constants ----
            ident_f = cpool.tile([128, 128], f32)
            make_identity(nc, ident_f[:])
            ident_bf = cpool.tile([128, 128], bf16)
            nc.vector.tensor_copy(ident_bf[:], ident_f[:])
            mask_f = cpool.tile([128, 128], f32)
            nc.gpsimd.memset(mask_f[:], 0.0)
            nc.gpsimd.affine_select(
                out=mask_f[:], in_=mask_f[:], compare_op=ALU.is_ge,
                fill=-30000.0, base=0, pattern=[[-1, 128]], channel_multiplier=1)
            mask_bf = cpool.tile([128, 128], bf16)
            nc.vector.tensor_copy(mask_bf[:], mask_f[:])
            ident_h = cpool.tile([128, 128], fp16)
            nc.vector.tensor_copy(ident_h[:], ident_f[:])
            mask_h = cpool.tile([128, 128], fp16)
            nc.vector.tensor_copy(mask_h[:], mask_f[:])
            ident_r = cpool.tile([128, 128], f32r)
            nc.vector.tensor_copy(ident_r[:], ident_f[:])
            mask_r = cpool.tile([128, 128], f32r)
            nc.vector.tensor_copy(mask_r[:], mask_f[:])
            # head-selector: sel[i, f] = 1 iff 64*i <= f < 64*i + 64
            sel_f = cpool.tile([2, 128], f32)
            nc.gpsimd.memset(sel_f[:], 1.0)
            nc.gpsimd.affine_select(
                out=sel_f[:], in_=sel_f[:], compare_op=ALU.is_ge, fill=0.0,
                base=0, pattern=[[1, 128]], channel_multiplier=-64)
            nc.gpsimd.affine_select(
                out=sel_f[:], in_=sel_f[:], compare_op=ALU.is_ge, fill=0.0,
                base=63, pattern=[[-1, 128]], channel_multiplier=64)
            sel_r = cpool.tile([2, 128], fp16)
            nc.vector.tensor_copy(sel_r[:], sel_f[:])

            # ---- weights ----
            wq_s = cpool.tile([128, NE, F], fp16)
            wk_s = cpool.tile([128, NE, F], fp16)
            wv_s = cpool.tile([128, NE, F], bf16)
            wot_s = cpool.tile([128, E], fp16)
            nc.sync.dma_start(wq_s[:], wq_d.rearrange("(n p) f -> p n f", p=128))
            nc.sync.dma_start(wk_s[:], wk_d.rearrange("(n p) f -> p n f", p=128))
            nc.sync.dma_start(wv_s[:], wv_d.rearrange("(n p) f -> p n f", p=128))
            nc.sync.dma_start(wot_s[:], wot_d)

            gctr = 0  # alternate ACT/DVE for PSUM evacuation copies

            def warm():
                # dummy weight loads keep the PE activity monitor from
                # re-throttling the clock during short gaps
                nc.tensor.ldweights(weights=ident_bf[:])

            for b in range(B):
                xt_s = xtp.tile([128, NE, T], fp16, name=f"xt_{b}", tag="xt")
                xtb_s = xtp.tile([128, NE, T], bf16, name=f"xtb_{b}", tag="xtb")
                for e in range(NE):
                    nc.sync.dma_start(
                        xt_s[:, e, :], xt_d[b, e * 128:(e + 1) * 128])
                    nc.sync.dma_start(
                        xtb_s[:, e, :], xtb_d[b, e * 128:(e + 1) * 128])

                # ---- projections: qT/kT [128, T] f32r, vT [128, T] bf16 ----
                qT = qkvp.tile([128, T], fp16, name=f"qT_{b}", tag="qT")
                kT = qkvp.tile([128, T], fp16, name=f"kT_{b}", tag="kT")
                vT = qkvp.tile([128, T], bf16, name=f"vT_{b}", tag="vT")
                for n in range(T // 512):
                    for w_s, dst in ((wq_s, qT), (wk_s, kT), (wv_s, vT)):
                        ps = ps_s.tile([128, 512], f32,
                                       name=f"prj_{b}_{n}_{dst.name}", tag="s")
                        x_src = xtb_s if dst is vT else xt_s
                        for e in range(NE):
                            nc.tensor.matmul(
                                ps[:], w_s[:, e, :],
                                x_src[:, e, n * 512:(n + 1) * 512],
                                start=(e == 0), stop=(e == NE - 1))
                        if dst is vT:
                            nc.vector.tensor_copy(
                                dst[:, n * 512:(n + 1) * 512], ps[:])
                        else:
                            nc.scalar.copy(
                                dst[:, n * 512:(n + 1) * 512], ps[:])

                # ---- V natural [128(u), NT, 128(f)] bf16 via PE transpose ----
                vn = qkvp.tile([128, NT, F], bf16, name=f"vn_{b}", tag="vn")
                for g in range(NT // 4):
                    vt_ps = ps_pt.tile([128, 512], bf16,
                                       name=f"vt_{b}_{g}", tag="pt")
                    for j in range(4):
                        u = g * 4 + j
                        nc.tensor.transpose(
                            vt_ps[:, j * 128:(j + 1) * 128],
                            vT[:, u * 128:(u + 1) * 128], ident_bf[:])
                    nc.vector.tensor_copy(
                        vn[:, g * 4:(g + 1) * 4, :].rearrange("p a b -> p (a b)"),
                        vt_ps[:])

                # ---- attention ----
                for tau in range(NT):
                    L = (tau + 1) * 128
                    nch = (L + 511) // 512
                    a_ps = ps_a.tile([128, 128], f32, name=f"a_{b}_{tau}", tag="a")
                    lboth = smallp.tile([128, 4], f32,
                                        name=f"lb_{b}_{tau}", tag="lb")
                    sml = {}
                    p_sb = {}
                    for h in range(HL):
                        sml[h] = smallp.tile([128, 16], f32,
                                             name=f"sml_{b}_{tau}_{h}", tag="sml")
                        p_sb[h] = ppool.tile([128, T], bf16,
                                             name=f"p_{b}_{tau}_{h}", tag="p")

                    # phase A: S chunks; per-chunk negated max while the
                    # chunks stay resident; exp with the row max afterwards
                    s_tiles = {}
                    for h in range(HL):
                        hs = slice(h * 64, (h + 1) * 64)
                        for c in range(nch):
                            c0 = c * 512
                            n = min(512, L - c0)
                            last = (c0 + n == L)
                            s_c = ps_s.tile([128, 512], f32,
                                            name=f"s_{b}_{tau}_{h}_{c}", tag="s")
                            s_tiles[(h, c)] = s_c
                            nc.tensor.matmul(
                                s_c[:, :n], qT[hs, tau * 128:(tau + 1) * 128],
                                kT[hs, c0:c0 + n], start=True, stop=not last)
                            if last:
                                nc.tensor.matmul(
                                    s_c[:, n - 128:n], ident_h[:], mask_h[:],
                                    start=False, stop=True)
                            nc.vector.reduce_max(
                                sml[h][:, c:c + 1], s_c[:, :n], axis=AX.X,
                                negate=True)

                        s_h = sml[h]
                        if nch == 1:
                            negm = s_h[:, 0:1]
                        else:
                            nc.vector.tensor_tensor(
                                s_h[:, 12:13], s_h[:, 0:1], s_h[:, 1:2],
                                op=ALU.min)
                            for c in range(2, nch):
                                nc.vector.tensor_tensor(
                                    s_h[:, 12:13], s_h[:, 12:13],
                                    s_h[:, c:c + 1], op=ALU.min)
                            negm = s_h[:, 12:13]
                        for c in range(nch):
                            c0 = c * 512
                            n = min(512, L - c0)
                            nc.scalar.activation(
                                p_sb[h][:, c0:c0 + n], s_tiles[(h, c)][:, :n],
                                AF.Exp, bias=negm, scale=1.0,
                                accum_out=s_h[:, 4 + c:5 + c])
                        if nch == 1:
                            nc.vector.tensor_copy(lboth[:, h:h + 1], s_h[:, 4:5])
                        else:
                            nc.vector.reduce_sum(
                                lboth[:, h:h + 1], s_h[:, 4:4 + nch], axis=AX.X)

                    # phase B per head: transposes + PV
                    pt_tiles = {}
                    for h in range(HL):
                        hs = slice(h * 64, (h + 1) * 64)
                        # transposes, grouped by 4 u-tiles
                        for g in range(nch):
                            nu = min(4, tau + 1 - g * 4)
                            rhs = ident_bf
                            pt_sb = ptsp.tile([128, 512], bf16,
                                              name=f"pt_{b}_{tau}_{h}_{g}",
                                              tag="pt")
                            pt_tiles[(h, g)] = pt_sb
                            pt_ps = ps_pt.tile([128, 512], bf16,
                                               name=f"ptp_{b}_{tau}_{h}_{g}",
                                               tag="pt")
                            for j in range(nu):
                                u = g * 4 + j
                                nc.tensor.transpose(
                                    pt_ps[:, j * 128:(j + 1) * 128],
                                    p_sb[h][:, u * 128:(u + 1) * 128], rhs[:])
                            gctr += 1
                            if gctr % 3 != 0:
                                nc.vector.tensor_copy(
                                    pt_sb[:, :nu * 128], pt_ps[:, :nu * 128])
                            else:
                                nc.scalar.copy(
                                    pt_sb[:, :nu * 128], pt_ps[:, :nu * 128])


                        # PV: A^T[f, t] += V[u, f].T @ P^T[u, t]
                        for u in range(tau + 1):
                            nc.tensor.matmul(
                                a_ps[hs, :], vn[:, u, hs],
                                pt_tiles[(h, u // 4)][:, (u % 4) * 128:(u % 4 + 1) * 128],
                                start=(u == 0), stop=(u == tau),
                                tile_position=(0, h * 64))

                    # ---- normalize A^T by 1/l (replicated via PE) ----
                    nc.vector.reciprocal(lboth[:, 2:4], lboth[:, 0:2])
                    rrep_ps = ps_o.tile([128, 128], f32,
                                        name=f"rr_{b}_{tau}", tag="o")
                    nc.tensor.transpose(rrep_ps[0:2, :], lboth[:, 2:4], ident_f[:])
                    rt_sb = smallp.tile([2, 128], fp16,
                                        name=f"rs_{b}_{tau}", tag="rs")
                    nc.vector.tensor_copy(rt_sb[:], rrep_ps[0:2, :])
                    nc.tensor.matmul(rrep_ps[:], sel_r[:], rt_sb[:],
                                     start=True, stop=True)
                    rrep_sb = smallp.tile([128, 128], f32,
                                          name=f"rb_{b}_{tau}", tag="rb")
                    nc.scalar.copy(rrep_sb[:], rrep_ps[:])
                    at_sb = smallp.tile([128, 128], fp16,
                                        name=f"at_{b}_{tau}", tag="at")
                    nc.vector.tensor_tensor(at_sb[:], a_ps[:], rrep_sb[:],
                                            op=ALU.mult)

                    # ---- out projection for this t-tile ----
                    out_sb = outp.tile([128, E], bf16,
                                       name=f"os_{b}_{tau}", tag="os")
                    for oc in range(2):
                        o_ps = ps_o.tile([128, 512], f32,
                                         name=f"o_{b}_{tau}_{oc}", tag="o")
                        nc.tensor.matmul(
                            o_ps[:], at_sb[:], wot_s[:, oc * 512:(oc + 1) * 512],
                            start=True, stop=True)
                        if oc == 0:
                            nc.vector.tensor_copy(
                                out_sb[:, oc * 512:(oc + 1) * 512], o_ps[:])
                        else:
                            nc.scalar.copy(
                                out_sb[:, oc * 512:(oc + 1) * 512], o_ps[:])
                    nc.sync.dma_start(
                        out_d[b, tau * 128:(tau + 1) * 128, :], out_sb[:])

    nc.compile()
    return nc


_NC_CACHE = None


def _get_nc():
    global _NC_CACHE
    if _NC_CACHE is None:
        _NC_CACHE = build_nc()
    return _NC_CACHE


def make_in_maps(x, Wq, Wk, Wv, Wo):
    x = np.asarray(x, np.float32)
    Wq = np.asarray(Wq, np.float32)
    Wk = np.asarray(Wk, np.float32)
    Wv = np.asarray(Wv, np.float32)
    Wo = np.asarray(Wo, np.float32)
    xtr = np.ascontiguousarray(x.transpose(0, 2, 1))  # [B, E, T]
    xt = xtr.astype(np.float16)
    import ml_dtypes
    xtb = xtr.astype(ml_dtypes.bfloat16)
    in_maps = []
    for c in range(N_CORES):
        h0 = c * HL
        wq = (np.concatenate([Wq[h0 + i] for i in range(HL)], axis=1)
              * np.float32(INV_S)).astype(np.float16)
        wk = np.concatenate([Wk[h0 + i] for i in range(HL)],
                            axis=1).astype(np.float16)
        import ml_dtypes
        wv = np.concatenate([Wv[h0 + i] for i in range(HL)],
                            axis=1).astype(ml_dtypes.bfloat16)
        wot = np.ascontiguousarray(
            Wo[:, c * F:(c + 1) * F].T).astype(np.float16)
        in_maps.append({"xt": xt, "xtb": xtb, "wq": wq, "wk": wk, "wv": wv,
                        "wot": wot})
    return in_maps


def run_on_cores(in_maps, trace=False, **kw):
    nc = _get_nc()
    return bass_utils.run_bass_kernel_spmd(
        nc, in_maps, core_ids=list(range(N_CORES)), trace=trace, **kw)


def kernel(x, mask, Wq, Wk, Wv, Wo):
    in_maps = make_in_maps(x, Wq, Wk, Wv, Wo)
    res = run_on_cores(in_maps)
    acc = np.zeros((B, T, E), np.float32)
    for c in range(N_CORES):
        acc += np.asarray(res.results[c]["out"], dtype=np.float32)
    return acc
